# revision 19
# baseline (speedup 1.0000x reference)
# Multi-head attention (N=2, S=2048, E=2048, H=16, Dk=128) on 8 NeuronCores.
#
# Sharding: 2 batches x 16 heads = 32 (n,h) pairs -> core c owns batch c//4,
# heads (c%4)*4 .. +4. The reference reshapes (N,H,S,Dk)->(N,S,H*Dk) without
# a head transpose, so rows [h*128,(h+1)*128) of the pre-projection matrix X
# (and hence of the final output) depend on head h only: each core computes
# 512 disjoint output rows and the host concatenates. No collectives.
#
# Device math per core (all matmuls fp32r, transposed layouts):
#   qT_c = Wq_c @ query[n].T   (hd x S)   kT_c same     v_c = value[n] @ Wv_c.T (S x hd)
#   sT   = k_h^T-tiles @ qT_h  (t x s)    expT = exp(sT/sqrt(Dk))
#   outT = v_h-tiles.T @ expT  (d x s)    denom: in-place add-tree + ones-matmul
#   out  = X_h @ Wo.T + bo  with X_h^T k-tiles = strided views of outT
#
# Perf notes (v3):
#  - all matmuls N=512 so the per-matmul fp32 weight load hides under the
#    ~225ns stream; x inputs are host-pretiled so every DMA reads a
#    contiguous 256KB block; weight DMAs are per-k so the first matmul
#    only waits for one tile.
#  - DMA issue (~0.6-0.8us sequencer time each) is spread across the
#    sync/gpsimd/scalar/vector queues; nothing is DMA-triggered from the
#    Scalar queue while it runs exps.
#  - v-projection is interleaved with the first two head-0 score chunks
#    so the Scalar engine (exp) starts early; exp runs on 2-bank psum
#    pairs (one (128,1024) op per two score tiles).
#  - softmax denominator: in-place add-tree on GpSimd (idle otherwise)
#    + ones-matmul column sum + PE partition-broadcast + fast reciprocal.
#  - outT spills to DRAM between phases to stay under the SBUF cap.
import numpy as np

D_MODEL = 2048
NHEAD = 16
DK = 128
N_BATCH = 2
SEQ = 2048
N_CORES = 8
HEADS_PER_CORE = 4


class Cfg:
    def __init__(self, S=SEQ, E=D_MODEL, NH=HEADS_PER_CORE, CH=512):
        assert S % 128 == 0 and E % 128 == 0
        self.S = S          # sequence length
        self.E = E          # model dim (contraction for projections)
        self.NH = NH        # heads per core
        self.CH = CH        # s-chunk width for attention phase
        self.NK = E // 128  # contraction tiles for projections / O-proj
        self.NT = S // 128  # t tiles (attention contraction)
        self.HDc = NH * DK  # head dims per core
        self.RPH = (S * DK) // E  # output rows per head (=128 at full size)
        assert self.RPH == 128, "O-proj layout assumes 128 rows per head"
        self.NCH = S // CH  # number of s-chunks
        assert S % CH == 0 and CH >= 256  # fp32r full-rate needs N>=256
        self.PCH = 512      # projection / O-proj free-dim chunk
        self.NPC = S // self.PCH   # projection s-chunks
        self.NOC = E // self.PCH   # O-proj output chunks


def build_program(cfg: Cfg):
    import concourse.bass as bass
    import concourse.tile as tile
    from concourse import bacc, mybir
    from contextlib import ExitStack

    fp32 = mybir.dt.float32
    fp32r = mybir.dt.float32r
    AF = mybir.ActivationFunctionType

    S, E, NH, CH = cfg.S, cfg.E, cfg.NH, cfg.CH
    NK, NT, HDc = cfg.NK, cfg.NT, cfg.HDc
    PCH, NPC, NOC, NCH = cfg.PCH, cfg.NPC, cfg.NOC, cfg.NCH
    inv_sqrt_dk = 1.0 / float(np.sqrt(DK))

    nc = bacc.Bacc("TRN2", target_bir_lowering=False, debug=False,
                   num_devices=N_CORES)

    # DRAM I/O (per-core values supplied via in_maps).
    # x inputs are host-pretiled: [k, s_chunk, partition, col].
    xqT = nc.dram_tensor("xqT", [NK, NPC, 128, PCH], fp32r,
                         kind="ExternalInput").ap()
    xkT = nc.dram_tensor("xkT", [NK, NPC, 128, PCH], fp32r,
                         kind="ExternalInput").ap()
    xvT = nc.dram_tensor("xvT", [NK, NPC, 128, PCH], fp32r,
                         kind="ExternalInput").ap()
    wqT = nc.dram_tensor("wqT", [E, HDc], fp32r, kind="ExternalInput").ap()
    wkT = nc.dram_tensor("wkT", [E, HDc], fp32r, kind="ExternalInput").ap()
    wvT = nc.dram_tensor("wvT", [E, HDc], fp32r, kind="ExternalInput").ap()
    woT = nc.dram_tensor("woT", [E, E], fp32r, kind="ExternalInput").ap()
    bq = nc.dram_tensor("bq", [128, NH], fp32, kind="ExternalInput").ap()
    bk = nc.dram_tensor("bk", [128, NH], fp32, kind="ExternalInput").ap()
    bv = nc.dram_tensor("bv", [1, HDc], fp32r, kind="ExternalInput").ap()
    bo = nc.dram_tensor("bo", [1, E], fp32r, kind="ExternalInput").ap()
    ones_d = nc.dram_tensor("ones", [128, 128], fp32r, kind="ExternalInput").ap()
    out = nc.dram_tensor("out", [NH * 128, E], fp32, kind="ExternalOutput").ap()
    # outT spill buffer between attention and O-projection
    ocd = nc.dram_tensor("ocd", [NH, 128, S], fp32r).ap()

    with tile.TileContext(nc) as tc, ExitStack() as ctx:
        consts = ctx.enter_context(tc.tile_pool(name="consts", bufs=1))
        ones_sb = consts.tile([128, 128], fp32r)
        nc.scalar.dma_start(ones_sb[:], ones_d)
        ones_col = ones_sb[:, :1]
        ones_row = ones_sb[:1, :]
        bq_sb = consts.tile([128, NH], fp32)
        bk_sb = consts.tile([128, NH], fp32)
        bv_sb = consts.tile([1, HDc], fp32r)
        nc.scalar.dma_start(bq_sb[:], bq)
        nc.scalar.dma_start(bk_sb[:], bk)
        nc.scalar.dma_start(bv_sb[:], bv)

        with tc.tile_pool(name="persist", bufs=1) as persist, \
             tc.tile_pool(name="xin", bufs=6) as xin:
            qc = persist.tile([128, NH, S], fp32r)    # qT_c: [d, h, s]
            kc = persist.tile([128, NH, S], fp32r)    # kT_c: [d, h, s]
            vc = persist.tile([128, NT, HDc], fp32r)  # v_c: [t_p, t_t, h*128+d]

            # ============== Phase A: q/k projections ==============
            with tc.tile_pool(name="wpool", bufs=1) as wpool, \
                 tc.tile_pool(name="pa_psum", bufs=2, space="PSUM") as pa:

                def proj_qk(w_dram, x_dram, bias_sb, dst, engs):
                    # dst[:, m, s*] = W_c @ x^T  (hd x S), bias fused in evict
                    w_sb = wpool.tile([128, NK, HDc], fp32r, tag="w",
                                      name="w_sb")
                    wd = w_dram.rearrange("(k p) c -> k p c", p=128)
                    for k in range(NK):
                        nc.scalar.dma_start(w_sb[:, k, :], wd[k])
                    for s in range(NPC):
                        ps = [pa.tile([128, PCH], fp32, tag=f"pa{m}",
                                      name=f"pa{m}") for m in range(NH)]
                        for k in range(NK):
                            xtile = xin.tile([128, PCH], fp32r, tag="xin")
                            engs[k % len(engs)].dma_start(
                                xtile[:], x_dram[k, s])
                            for m in range(NH):
                                nc.tensor.matmul(
                                    ps[m][:],
                                    w_sb[:, k, m * 128:(m + 1) * 128],
                                    xtile[:], start=(k == 0),
                                    stop=(k == NK - 1))
                        for m in range(NH):
                            nc.vector.tensor_scalar_add(
                                dst[:, m, s * PCH:(s + 1) * PCH], ps[m][:],
                                bias_sb[:, m:m + 1])

                proj_qk(wqT, xqT, bq_sb, qc, [nc.sync, nc.scalar])
                proj_qk(wkT, xkT, bk_sb, kc, [nc.gpsimd])

            # ====== Phase B: v-projection interleaved with attention ======
            wvd = wvT.rearrange("(k p) c -> k p c", p=128)
            with tc.tile_pool(name="wvp", bufs=4) as wvp, \
                 tc.tile_pool(name="expp", bufs=2) as expp, \
                 tc.tile_pool(name="bsc", bufs=2) as bsc, \
                 tc.tile_pool(name="ocsb", bufs=2) as ocsb, \
                 tc.tile_pool(name="st_psum", bufs=2, space="PSUM") as stp:

                def proj_v_group(tc4, pav):
                    # 4 t-tiles of v: stationary = xvT tile slices, rhs = w
                    ps = [pav.tile([128, HDc], fp32, tag=f"pav{j}",
                                   name=f"pav{j}") for j in range(4)]
                    for k in range(NK):
                        xtile = xin.tile([128, PCH], fp32r, tag="xin")
                        nc.gpsimd.dma_start(xtile[:], xvT[k, tc4])
                        wtile = wvp.tile([128, HDc], fp32r, tag="wv",
                                         name="wv")
                        nc.sync.dma_start(wtile[:], wvd[k])
                        for j in range(4):
                            nc.tensor.matmul(
                                ps[j][:], xtile[:, j * 128:(j + 1) * 128],
                                wtile[:], start=(k == 0), stop=False)
                    for j in range(4):
                        nc.tensor.matmul(ps[j][:], ones_row, bv_sb[:],
                                         start=False, stop=True)
                        nc.vector.tensor_copy(vc[:, tc4 * 4 + j, :], ps[j][:])

                def scores_exp(h, c):
                    # paired score tiles in a 2-bank psum; one exp per pair
                    cs = slice(c * CH, (c + 1) * CH)
                    expT = expp.tile([128, NT, CH], fp32r, tag="expT",
                                     name=f"expT_{h}_{c}")
                    for i in range(NT // 2):
                        ps = stp.tile([128, 2 * CH], fp32, tag="st", name="st")
                        for half in range(2):
                            tt = 2 * i + half
                            nc.tensor.matmul(
                                ps[:, half * CH:(half + 1) * CH],
                                kc[:, h, tt * 128:(tt + 1) * 128],
                                qc[:, h, cs], start=True, stop=True)
                        nc.scalar.activation(
                            expT[:, 2 * i:2 * i + 2, :],
                            ps[:].rearrange("p (a b) -> p a b", a=2),
                            AF.Exp, scale=inv_sqrt_dk)
                    return expT

                def attn_tail(h, c, expT, otp, dnp):
                    cs = slice(c * CH, (c + 1) * CH)
                    # outT and denominator column-sum interleaved; both are
                    # PSUM-accumulated matmul chains over the 16 t-tiles
                    op = otp.tile([128, CH], fp32, tag="ot", name="ot")
                    dn = dnp.tile([1, CH], fp32, tag="dn", name="dn")
                    for tt in range(NT):
                        nc.tensor.matmul(
                            op[:], vc[:, tt, h * 128:(h + 1) * 128],
                            expT[:, tt, :], start=(tt == 0),
                            stop=(tt == NT - 1))
                        nc.tensor.matmul(
                            dn[:], ones_col, expT[:, tt, :],
                            start=(tt == 0), stop=(tt == NT - 1))
                    dn_sb = bsc.tile([1, CH], fp32r, tag="dnsb", name="dnsb")
                    nc.vector.tensor_copy(dn_sb[:], dn[:])
                    dbc = dnp.tile([128, CH], fp32, tag="dn", name="dbc")
                    nc.tensor.matmul(dbc[:], ones_row, dn_sb[:],
                                     start=True, stop=True)
                    rsc = bsc.tile([128, CH], fp32, tag="rsc", name="rsc")
                    nc.vector.reciprocal_approx_fast(rsc[:], dbc[:])
                    oc_t = ocsb.tile([128, CH], fp32r, tag="oct", name="oct")
                    nc.vector.tensor_mul(oc_t[:], op[:], rsc[:])
                    nc.sync.dma_start(ocd[h, :, cs], oc_t[:])

                pairs = [(h, c) for h in range(NH) for c in range(NCH)]
                pend = []
                si = 0
                # v-groups interleaved with the first two score chunks (the
                # expT double-buffer limit); tails only after all of vc
                with tc.tile_pool(name="pav_psum", bufs=1,
                                  space="PSUM") as pav:
                    for g in range(NT // 4):
                        proj_v_group(g, pav)
                        if si < len(pairs) and len(pend) < 2:
                            pend.append((*pairs[si], scores_exp(*pairs[si])))
                            si += 1
                with tc.tile_pool(name="ot_psum", bufs=2,
                                  space="PSUM") as otp, \
                     tc.tile_pool(name="dn_psum", bufs=2,
                                  space="PSUM") as dnp:
                    while si < len(pairs) and len(pend) < 2:
                        pend.append((*pairs[si], scores_exp(*pairs[si])))
                        si += 1
                    while si < len(pairs):
                        attn_tail(*pend.pop(0), otp, dnp)
                        pend.append((*pairs[si], scores_exp(*pairs[si])))
                        si += 1
                    while pend:
                        attn_tail(*pend.pop(0), otp, dnp)

        # ============== Phase C: output projection ==============
        with tc.tile_pool(name="ocin", bufs=1) as ocin, \
             tc.tile_pool(name="wo_in", bufs=6) as wo_in, \
             tc.tile_pool(name="osb", bufs=8) as osb, \
             tc.tile_pool(name="pc_psum", bufs=2, space="PSUM") as pc:
            bo_sb = ocin.tile([1, E], fp32r, name="bo_sb")
            nc.scalar.dma_start(bo_sb[:], bo)
            oc_h = []
            for h in range(NH):
                t = ocin.tile([128, S], fp32r, name=f"ocin{h}")
                nc.scalar.dma_start(t[:], ocd[h])
                oc_h.append(t)
            wot = woT.rearrange("(k p) e -> k p e", p=128)
            for nn in range(NOC):
                ns = slice(nn * PCH, (nn + 1) * PCH)
                ps = [pc.tile([128, PCH], fp32, tag=f"pc{h}", name=f"pc{h}")
                      for h in range(NH)]
                for k in range(NK):
                    wtile = wo_in.tile([128, PCH], fp32r, tag="wo")
                    eng = nc.sync if k % 2 == 0 else nc.gpsimd
                    eng.dma_start(wtile[:], wot[k, :, ns])
                    for h in range(NH):
                        # lhsT = X_h^T k-tile: strided view of outT
                        lhs = oc_h[h].rearrange(
                            "p (j i) -> p i j", i=NK)[:, k, :]
                        nc.tensor.matmul(ps[h][:], lhs, wtile[:],
                                         start=(k == 0), stop=False)
                for h in range(NH):
                    nc.tensor.matmul(ps[h][:], ones_row, bo_sb[:, ns],
                                     start=False, stop=True)
                    ot = osb.tile([128, PCH], fp32, tag="osb")
                    nc.vector.tensor_copy(ot[:], ps[h][:])
                    nc.sync.dma_start(out[h * 128:(h + 1) * 128, ns], ot[:])

    nc.compile()
    return nc


def _tile_x(xt, NK, NPC, PCH):
    # (E, S) -> [k, s_chunk, partition, col] contiguous
    return np.ascontiguousarray(
        xt.reshape(NK, 128, NPC, PCH).transpose(0, 2, 1, 3))


def shard_inputs(cfg: Cfg, query, key, value, Wq, bq, Wk, bk, Wv, bv, Wo, bo):
    """Build per-core in_maps from full inputs."""
    f = np.float32
    query, key, value = (np.asarray(a, f) for a in (query, key, value))
    Wq, Wk, Wv, Wo = (np.asarray(a, f) for a in (Wq, Wk, Wv, Wo))
    bq, bk, bv, bo = (np.asarray(a, f) for a in (bq, bk, bv, bo))
    NH, HDc, NK, NPC, PCH = cfg.NH, cfg.HDc, cfg.NK, cfg.NPC, cfg.PCH
    woT = np.ascontiguousarray(Wo.T)
    _ONES = np.ones((128, 128), np.float32)
    bo_r = np.ascontiguousarray(bo.reshape(1, -1))
    xq_t = [_tile_x(query[n].T, NK, NPC, PCH) for n in range(N_BATCH)]
    xk_t = [_tile_x(key[n].T, NK, NPC, PCH) for n in range(N_BATCH)]
    xv_t = [_tile_x(value[n].T, NK, NPC, PCH) for n in range(N_BATCH)]
    in_maps = []
    cores_per_batch = N_CORES // N_BATCH
    for c in range(N_CORES):
        n = c // cores_per_batch
        hs = (c % cores_per_batch) * HDc
        sl = slice(hs, hs + HDc)
        in_maps.append({
            "xqT": xq_t[n],
            "xkT": xk_t[n],
            "xvT": xv_t[n],
            "wqT": np.ascontiguousarray(Wq[sl].T),
            "wkT": np.ascontiguousarray(Wk[sl].T),
            "wvT": np.ascontiguousarray(Wv[sl].T),
            "woT": woT,
            "bq": np.ascontiguousarray(bq[sl].reshape(NH, 128).T),
            "bk": np.ascontiguousarray(bk[sl].reshape(NH, 128).T),
            "bv": np.ascontiguousarray(bv[sl].reshape(1, HDc)),
            "bo": bo_r,
            "ones": _ONES,
        })
    return in_maps


def gather_outputs(cfg: Cfg, results):
    """results: list of per-core {'out': (NH*128, E)} -> full (N, S, E)."""
    E = cfg.E
    full = np.empty((N_BATCH, SEQ, E), np.float32)
    cores_per_batch = N_CORES // N_BATCH
    rows = cfg.NH * 128
    for c in range(N_CORES):
        n = c // cores_per_batch
        r0 = (c % cores_per_batch) * rows
        full[n, r0:r0 + rows, :] = results[c]["out"]
    return full


_CACHE = {}


def kernel(**inputs) -> np.ndarray:
    from concourse.bass_utils import run_bass_kernel_spmd
    cfg = Cfg()
    if "nc" not in _CACHE:
        _CACHE["nc"] = build_program(cfg)
    nc = _CACHE["nc"]
    in_maps = shard_inputs(cfg, **inputs)
    res = run_bass_kernel_spmd(nc, in_maps, core_ids=list(range(N_CORES)))
    return gather_outputs(cfg, res.results)


# revision 20
# speedup vs baseline: 1.0687x; 1.0687x over previous
# Multi-head attention (N=2, S=2048, E=2048, H=16, Dk=128) on 8 NeuronCores.
#
# Sharding: 2 batches x 16 heads = 32 (n,h) pairs -> core c owns batch c//4,
# heads (c%4)*4 .. +4. The reference reshapes (N,H,S,Dk)->(N,S,H*Dk) without
# a head transpose, so rows [h*128,(h+1)*128) of the pre-projection matrix X
# (and hence of the final output) depend on head h only: each core computes
# 512 disjoint output rows and the host concatenates. No collectives.
#
# Device math per core (all matmuls fp32r, transposed layouts):
#   qT_c = Wq_c @ query[n].T   (hd x S)   kT_c same     v_c = value[n] @ Wv_c.T (S x hd)
#   sT   = k_h^T-tiles @ qT_h  (t x s)    expT = exp(sT/sqrt(Dk))
#   outT = v_h-tiles.T @ expT  (d x s)    denom: in-place add-tree + ones-matmul
#   out  = X_h @ Wo.T + bo  with X_h^T k-tiles = strided views of outT
#
# Perf notes (v3):
#  - all matmuls N=512 so the per-matmul fp32 weight load hides under the
#    ~225ns stream; x inputs are host-pretiled so every DMA reads a
#    contiguous 256KB block; weight DMAs are per-k so the first matmul
#    only waits for one tile.
#  - DMA issue (~0.6-0.8us sequencer time each) is spread across the
#    sync/gpsimd/scalar/vector queues; nothing is DMA-triggered from the
#    Scalar queue while it runs exps.
#  - v-projection is interleaved with the first two head-0 score chunks
#    so the Scalar engine (exp) starts early; exp runs on 2-bank psum
#    pairs (one (128,1024) op per two score tiles).
#  - softmax denominator: in-place add-tree on GpSimd (idle otherwise)
#    + ones-matmul column sum + PE partition-broadcast + fast reciprocal.
#  - outT spills to DRAM between phases to stay under the SBUF cap.
import numpy as np

D_MODEL = 2048
NHEAD = 16
DK = 128
N_BATCH = 2
SEQ = 2048
N_CORES = 8
HEADS_PER_CORE = 4


class Cfg:
    def __init__(self, S=SEQ, E=D_MODEL, NH=HEADS_PER_CORE, CH=512):
        assert S % 128 == 0 and E % 128 == 0
        self.S = S          # sequence length
        self.E = E          # model dim (contraction for projections)
        self.NH = NH        # heads per core
        self.CH = CH        # s-chunk width for attention phase
        self.NK = E // 128  # contraction tiles for projections / O-proj
        self.NT = S // 128  # t tiles (attention contraction)
        self.HDc = NH * DK  # head dims per core
        self.RPH = (S * DK) // E  # output rows per head (=128 at full size)
        assert self.RPH == 128, "O-proj layout assumes 128 rows per head"
        self.NCH = S // CH  # number of s-chunks
        assert S % CH == 0 and CH >= 256  # fp32r full-rate needs N>=256
        self.PCH = 512      # projection / O-proj free-dim chunk
        self.NPC = S // self.PCH   # projection s-chunks
        self.NOC = E // self.PCH   # O-proj output chunks


def build_program(cfg: Cfg):
    import concourse.bass as bass
    import concourse.tile as tile
    from concourse import bacc, mybir
    from contextlib import ExitStack

    fp32 = mybir.dt.float32
    fp32r = mybir.dt.float32r
    AF = mybir.ActivationFunctionType

    S, E, NH, CH = cfg.S, cfg.E, cfg.NH, cfg.CH
    NK, NT, HDc = cfg.NK, cfg.NT, cfg.HDc
    PCH, NPC, NOC, NCH = cfg.PCH, cfg.NPC, cfg.NOC, cfg.NCH
    inv_sqrt_dk = 1.0 / float(np.sqrt(DK))

    nc = bacc.Bacc("TRN2", target_bir_lowering=False, debug=False,
                   num_devices=N_CORES)

    # DRAM I/O (per-core values supplied via in_maps).
    # x inputs are host-pretiled: [k, s_chunk, partition, col].
    xqT = nc.dram_tensor("xqT", [NK, NPC, 128, PCH], fp32r,
                         kind="ExternalInput").ap()
    xkT = nc.dram_tensor("xkT", [NK, NPC, 128, PCH], fp32r,
                         kind="ExternalInput").ap()
    xvT = nc.dram_tensor("xvT", [NK, NPC, 128, PCH], fp32r,
                         kind="ExternalInput").ap()
    wqT = nc.dram_tensor("wqT", [E, HDc], fp32r, kind="ExternalInput").ap()
    wkT = nc.dram_tensor("wkT", [E, HDc], fp32r, kind="ExternalInput").ap()
    wvT = nc.dram_tensor("wvT", [E, HDc], fp32r, kind="ExternalInput").ap()
    woT = nc.dram_tensor("woT", [E, E], fp32r, kind="ExternalInput").ap()
    bq = nc.dram_tensor("bq", [128, NH], fp32, kind="ExternalInput").ap()
    bk = nc.dram_tensor("bk", [128, NH], fp32, kind="ExternalInput").ap()
    bv = nc.dram_tensor("bv", [1, HDc], fp32r, kind="ExternalInput").ap()
    bo = nc.dram_tensor("bo", [1, E], fp32r, kind="ExternalInput").ap()
    ones_d = nc.dram_tensor("ones", [128, 128], fp32r, kind="ExternalInput").ap()
    out = nc.dram_tensor("out", [NH * 128, E], fp32, kind="ExternalOutput").ap()
    # outT spill buffer between attention and O-projection
    ocd = nc.dram_tensor("ocd", [NH, 128, S], fp32r).ap()

    with tile.TileContext(nc) as tc, ExitStack() as ctx:
        consts = ctx.enter_context(tc.tile_pool(name="consts", bufs=1))
        ones_sb = consts.tile([128, 128], fp32r)
        nc.scalar.dma_start(ones_sb[:], ones_d)
        ones_col = ones_sb[:, :1]
        ones_row = ones_sb[:1, :]
        bq_sb = consts.tile([128, NH], fp32)
        bk_sb = consts.tile([128, NH], fp32)
        bv_sb = consts.tile([1, HDc], fp32r)
        nc.scalar.dma_start(bq_sb[:], bq)
        nc.scalar.dma_start(bk_sb[:], bk)
        nc.scalar.dma_start(bv_sb[:], bv)

        with tc.tile_pool(name="persist", bufs=1) as persist, \
             tc.tile_pool(name="xin", bufs=6) as xin:
            qc = persist.tile([128, NH, S], fp32r)    # qT_c: [d, h, s]
            kc = persist.tile([128, NH, S], fp32r)    # kT_c: [d, h, s]
            vc = persist.tile([128, NT, HDc], fp32r)  # v_c: [t_p, t_t, h*128+d]

            # ============== Phase A: q/k projections ==============
            with tc.tile_pool(name="wpool", bufs=1) as wpool, \
                 tc.tile_pool(name="pa_psum", bufs=2, space="PSUM") as pa:

                def proj_qk(w_dram, x_dram, bias_sb, dst, engs):
                    # dst[:, m, s*] = W_c @ x^T  (hd x S), bias fused in evict
                    w_sb = wpool.tile([128, NK, HDc], fp32r, tag="w",
                                      name="w_sb")
                    wd = w_dram.rearrange("(k p) c -> k p c", p=128)
                    for k in range(NK):
                        nc.scalar.dma_start(w_sb[:, k, :], wd[k])
                    for s in range(NPC):
                        ps = [pa.tile([128, PCH], fp32, tag=f"pa{m}",
                                      name=f"pa{m}") for m in range(NH)]
                        for k in range(NK):
                            xtile = xin.tile([128, PCH], fp32r, tag="xin")
                            engs[k % len(engs)].dma_start(
                                xtile[:], x_dram[k, s])
                            for m in range(NH):
                                nc.tensor.matmul(
                                    ps[m][:],
                                    w_sb[:, k, m * 128:(m + 1) * 128],
                                    xtile[:], start=(k == 0),
                                    stop=(k == NK - 1))
                        for m in range(NH):
                            nc.vector.tensor_scalar_add(
                                dst[:, m, s * PCH:(s + 1) * PCH], ps[m][:],
                                bias_sb[:, m:m + 1])

                proj_qk(wqT, xqT, bq_sb, qc, [nc.sync, nc.scalar])
                proj_qk(wkT, xkT, bk_sb, kc, [nc.gpsimd])

            # ====== Phase B: v-projection interleaved with attention ======
            wvd = wvT.rearrange("(k p) c -> k p c", p=128)
            with tc.tile_pool(name="wvp", bufs=4) as wvp, \
                 tc.tile_pool(name="expp", bufs=2) as expp, \
                 tc.tile_pool(name="bsc", bufs=2) as bsc, \
                 tc.tile_pool(name="ocsb", bufs=2) as ocsb, \
                 tc.tile_pool(name="st_psum", bufs=4, space="PSUM") as stp:

                def proj_v_group(tc4, pav):
                    # 4 t-tiles of v: stationary = xvT tile slices, rhs = w
                    ps = [pav.tile([128, HDc], fp32, tag=f"pav{j}",
                                   name=f"pav{j}") for j in range(4)]
                    for k in range(NK):
                        xtile = xin.tile([128, PCH], fp32r, tag="xin")
                        nc.gpsimd.dma_start(xtile[:], xvT[k, tc4])
                        wtile = wvp.tile([128, HDc], fp32r, tag="wv",
                                         name="wv")
                        nc.sync.dma_start(wtile[:], wvd[k])
                        for j in range(4):
                            nc.tensor.matmul(
                                ps[j][:], xtile[:, j * 128:(j + 1) * 128],
                                wtile[:], start=(k == 0), stop=False)
                    for j in range(4):
                        nc.tensor.matmul(ps[j][:], ones_row, bv_sb[:],
                                         start=False, stop=True)
                        nc.vector.tensor_copy(vc[:, tc4 * 4 + j, :], ps[j][:])

                def scores_exp(h, c):
                    cs = slice(c * CH, (c + 1) * CH)
                    expT = expp.tile([128, NT, CH], fp32r, tag="expT",
                                     name=f"expT_{h}_{c}")
                    for tt in range(NT):
                        ps = stp.tile([128, CH], fp32, tag="st", name="st")
                        nc.tensor.matmul(
                            ps[:], kc[:, h, tt * 128:(tt + 1) * 128],
                            qc[:, h, cs], start=True, stop=True)
                        nc.scalar.activation(expT[:, tt, :], ps[:],
                                             AF.Exp, scale=inv_sqrt_dk)
                    return expT

                def attn_tail(h, c, expT, otp, dnp):
                    cs = slice(c * CH, (c + 1) * CH)
                    # outT and denominator column-sum interleaved; both are
                    # PSUM-accumulated matmul chains over the 16 t-tiles
                    op = otp.tile([128, CH], fp32, tag="ot", name="ot")
                    dn = dnp.tile([1, CH], fp32, tag="dn", name="dn")
                    for tt in range(NT):
                        nc.tensor.matmul(
                            op[:], vc[:, tt, h * 128:(h + 1) * 128],
                            expT[:, tt, :], start=(tt == 0),
                            stop=(tt == NT - 1))
                        nc.tensor.matmul(
                            dn[:], ones_col, expT[:, tt, :],
                            start=(tt == 0), stop=(tt == NT - 1))
                    dn_sb = bsc.tile([1, CH], fp32r, tag="dnsb", name="dnsb")
                    nc.vector.tensor_copy(dn_sb[:], dn[:])
                    dbc = dnp.tile([128, CH], fp32, tag="dn", name="dbc")
                    nc.tensor.matmul(dbc[:], ones_row, dn_sb[:],
                                     start=True, stop=True)
                    rsc = bsc.tile([128, CH], fp32, tag="rsc", name="rsc")
                    nc.vector.reciprocal_approx_fast(rsc[:], dbc[:])
                    oc_t = ocsb.tile([128, CH], fp32r, tag="oct", name="oct")
                    nc.vector.tensor_mul(oc_t[:], op[:], rsc[:])
                    nc.sync.dma_start(ocd[h, :, cs], oc_t[:])

                pairs = [(h, c) for h in range(NH) for c in range(NCH)]
                pend = []
                si = 0
                # v-groups interleaved with the first two score chunks (the
                # expT double-buffer limit); tails only after all of vc
                with tc.tile_pool(name="pav_psum", bufs=1,
                                  space="PSUM") as pav:
                    for g in range(NT // 4):
                        proj_v_group(g, pav)
                        if si < len(pairs) and len(pend) < 2:
                            pend.append((*pairs[si], scores_exp(*pairs[si])))
                            si += 1
                with tc.tile_pool(name="ot_psum", bufs=2,
                                  space="PSUM") as otp, \
                     tc.tile_pool(name="dn_psum", bufs=2,
                                  space="PSUM") as dnp:
                    while si < len(pairs) and len(pend) < 2:
                        pend.append((*pairs[si], scores_exp(*pairs[si])))
                        si += 1
                    while si < len(pairs):
                        attn_tail(*pend.pop(0), otp, dnp)
                        pend.append((*pairs[si], scores_exp(*pairs[si])))
                        si += 1
                    while pend:
                        attn_tail(*pend.pop(0), otp, dnp)

        # ============== Phase C: output projection ==============
        with tc.tile_pool(name="ocin", bufs=1) as ocin, \
             tc.tile_pool(name="wo_in", bufs=6) as wo_in, \
             tc.tile_pool(name="osb", bufs=8) as osb, \
             tc.tile_pool(name="pc_psum", bufs=2, space="PSUM") as pc:
            bo_sb = ocin.tile([1, E], fp32r, name="bo_sb")
            nc.scalar.dma_start(bo_sb[:], bo)
            oc_h = []
            for h in range(NH):
                t = ocin.tile([128, S], fp32r, name=f"ocin{h}")
                nc.scalar.dma_start(t[:], ocd[h])
                oc_h.append(t)
            wot = woT.rearrange("(k p) e -> k p e", p=128)
            for nn in range(NOC):
                ns = slice(nn * PCH, (nn + 1) * PCH)
                ps = [pc.tile([128, PCH], fp32, tag=f"pc{h}", name=f"pc{h}")
                      for h in range(NH)]
                for k in range(NK):
                    wtile = wo_in.tile([128, PCH], fp32r, tag="wo")
                    eng = nc.sync if k % 2 == 0 else nc.gpsimd
                    eng.dma_start(wtile[:], wot[k, :, ns])
                    for h in range(NH):
                        # lhsT = X_h^T k-tile: strided view of outT
                        lhs = oc_h[h].rearrange(
                            "p (j i) -> p i j", i=NK)[:, k, :]
                        nc.tensor.matmul(ps[h][:], lhs, wtile[:],
                                         start=(k == 0), stop=False)
                for h in range(NH):
                    nc.tensor.matmul(ps[h][:], ones_row, bo_sb[:, ns],
                                     start=False, stop=True)
                    ot = osb.tile([128, PCH], fp32, tag="osb")
                    nc.vector.tensor_copy(ot[:], ps[h][:])
                    nc.sync.dma_start(out[h * 128:(h + 1) * 128, ns], ot[:])

    nc.compile()
    return nc


def _tile_x(xt, NK, NPC, PCH):
    # (E, S) -> [k, s_chunk, partition, col] contiguous
    return np.ascontiguousarray(
        xt.reshape(NK, 128, NPC, PCH).transpose(0, 2, 1, 3))


def shard_inputs(cfg: Cfg, query, key, value, Wq, bq, Wk, bk, Wv, bv, Wo, bo):
    """Build per-core in_maps from full inputs."""
    f = np.float32
    query, key, value = (np.asarray(a, f) for a in (query, key, value))
    Wq, Wk, Wv, Wo = (np.asarray(a, f) for a in (Wq, Wk, Wv, Wo))
    bq, bk, bv, bo = (np.asarray(a, f) for a in (bq, bk, bv, bo))
    NH, HDc, NK, NPC, PCH = cfg.NH, cfg.HDc, cfg.NK, cfg.NPC, cfg.PCH
    woT = np.ascontiguousarray(Wo.T)
    _ONES = np.ones((128, 128), np.float32)
    bo_r = np.ascontiguousarray(bo.reshape(1, -1))
    xq_t = [_tile_x(query[n].T, NK, NPC, PCH) for n in range(N_BATCH)]
    xk_t = [_tile_x(key[n].T, NK, NPC, PCH) for n in range(N_BATCH)]
    xv_t = [_tile_x(value[n].T, NK, NPC, PCH) for n in range(N_BATCH)]
    in_maps = []
    cores_per_batch = N_CORES // N_BATCH
    for c in range(N_CORES):
        n = c // cores_per_batch
        hs = (c % cores_per_batch) * HDc
        sl = slice(hs, hs + HDc)
        in_maps.append({
            "xqT": xq_t[n],
            "xkT": xk_t[n],
            "xvT": xv_t[n],
            "wqT": np.ascontiguousarray(Wq[sl].T),
            "wkT": np.ascontiguousarray(Wk[sl].T),
            "wvT": np.ascontiguousarray(Wv[sl].T),
            "woT": woT,
            "bq": np.ascontiguousarray(bq[sl].reshape(NH, 128).T),
            "bk": np.ascontiguousarray(bk[sl].reshape(NH, 128).T),
            "bv": np.ascontiguousarray(bv[sl].reshape(1, HDc)),
            "bo": bo_r,
            "ones": _ONES,
        })
    return in_maps


def gather_outputs(cfg: Cfg, results):
    """results: list of per-core {'out': (NH*128, E)} -> full (N, S, E)."""
    E = cfg.E
    full = np.empty((N_BATCH, SEQ, E), np.float32)
    cores_per_batch = N_CORES // N_BATCH
    rows = cfg.NH * 128
    for c in range(N_CORES):
        n = c // cores_per_batch
        r0 = (c % cores_per_batch) * rows
        full[n, r0:r0 + rows, :] = results[c]["out"]
    return full


_CACHE = {}


def kernel(**inputs) -> np.ndarray:
    from concourse.bass_utils import run_bass_kernel_spmd
    cfg = Cfg()
    if "nc" not in _CACHE:
        _CACHE["nc"] = build_program(cfg)
    nc = _CACHE["nc"]
    in_maps = shard_inputs(cfg, **inputs)
    res = run_bass_kernel_spmd(nc, in_maps, core_ids=list(range(N_CORES)))
    return gather_outputs(cfg, res.results)


# revision 21
# speedup vs baseline: 1.1761x; 1.1005x over previous
# Multi-head attention (N=2, S=2048, E=2048, H=16, Dk=128) on 8 NeuronCores.
#
# Sharding: 2 batches x 16 heads = 32 (n,h) pairs -> core c owns batch c//4,
# heads (c%4)*4 .. +4. The reference reshapes (N,H,S,Dk)->(N,S,H*Dk) without
# a head transpose, so rows [h*128,(h+1)*128) of the pre-projection matrix X
# (and hence of the final output) depend on head h only: each core computes
# 512 disjoint output rows and the host concatenates. No collectives.
#
# Device math per core (all matmuls fp32r, transposed layouts):
#   qT_c = Wq_c @ query[n].T   (hd x S)   kT_c same     v_c = value[n] @ Wv_c.T (S x hd)
#   sT   = k_h^T-tiles @ qT_h  (t x s)    expT = exp(sT/sqrt(Dk))
#   outT = v_h-tiles.T @ expT  (d x s)    denom: in-place add-tree + ones-matmul
#   out  = X_h @ Wo.T + bo  with X_h^T k-tiles = strided views of outT
#
# Perf notes (v3):
#  - all matmuls N=512 so the per-matmul fp32 weight load hides under the
#    ~225ns stream; x inputs are host-pretiled so every DMA reads a
#    contiguous 256KB block; weight DMAs are per-k so the first matmul
#    only waits for one tile.
#  - DMA issue (~0.6-0.8us sequencer time each) is spread across the
#    sync/gpsimd/scalar/vector queues; nothing is DMA-triggered from the
#    Scalar queue while it runs exps.
#  - v-projection is interleaved with the first two head-0 score chunks
#    so the Scalar engine (exp) starts early; exp runs on 2-bank psum
#    pairs (one (128,1024) op per two score tiles).
#  - softmax denominator: in-place add-tree on GpSimd (idle otherwise)
#    + ones-matmul column sum + PE partition-broadcast + fast reciprocal.
#  - outT spills to DRAM between phases to stay under the SBUF cap.
import numpy as np

D_MODEL = 2048
NHEAD = 16
DK = 128
N_BATCH = 2
SEQ = 2048
N_CORES = 8
HEADS_PER_CORE = 4


class Cfg:
    def __init__(self, S=SEQ, E=D_MODEL, NH=HEADS_PER_CORE, CH=512):
        assert S % 128 == 0 and E % 128 == 0
        self.S = S          # sequence length
        self.E = E          # model dim (contraction for projections)
        self.NH = NH        # heads per core
        self.CH = CH        # s-chunk width for attention phase
        self.NK = E // 128  # contraction tiles for projections / O-proj
        self.NT = S // 128  # t tiles (attention contraction)
        self.HDc = NH * DK  # head dims per core
        self.RPH = (S * DK) // E  # output rows per head (=128 at full size)
        assert self.RPH == 128, "O-proj layout assumes 128 rows per head"
        self.NCH = S // CH  # number of s-chunks
        assert S % CH == 0 and CH >= 256  # fp32r full-rate needs N>=256
        self.PCH = 512      # projection / O-proj free-dim chunk
        self.NPC = S // self.PCH   # projection s-chunks
        self.NOC = E // self.PCH   # O-proj output chunks


def build_program(cfg: Cfg):
    import concourse.bass as bass
    import concourse.tile as tile
    from concourse import bacc, mybir
    from contextlib import ExitStack

    fp32 = mybir.dt.float32
    fp32r = mybir.dt.float32r
    AF = mybir.ActivationFunctionType

    S, E, NH, CH = cfg.S, cfg.E, cfg.NH, cfg.CH
    NK, NT, HDc = cfg.NK, cfg.NT, cfg.HDc
    PCH, NPC, NOC, NCH = cfg.PCH, cfg.NPC, cfg.NOC, cfg.NCH
    inv_sqrt_dk = 1.0 / float(np.sqrt(DK))

    nc = bacc.Bacc("TRN2", target_bir_lowering=False, debug=False,
                   num_devices=N_CORES)

    # DRAM I/O (per-core values supplied via in_maps).
    # x inputs are host-pretiled: [k, s_chunk, partition, col].
    xqT = nc.dram_tensor("xqT", [NK, NPC, 128, PCH], fp32r,
                         kind="ExternalInput").ap()
    xkT = nc.dram_tensor("xkT", [NK, NPC, 128, PCH], fp32r,
                         kind="ExternalInput").ap()
    xvT = nc.dram_tensor("xvT", [NK, NPC, 128, PCH], fp32r,
                         kind="ExternalInput").ap()
    wqT = nc.dram_tensor("wqT", [E, HDc], fp32r, kind="ExternalInput").ap()
    wkT = nc.dram_tensor("wkT", [E, HDc], fp32r, kind="ExternalInput").ap()
    wvT = nc.dram_tensor("wvT", [E, HDc], fp32r, kind="ExternalInput").ap()
    woT = nc.dram_tensor("woT", [E, E], fp32r, kind="ExternalInput").ap()
    bq = nc.dram_tensor("bq", [128, NH], fp32, kind="ExternalInput").ap()
    bk = nc.dram_tensor("bk", [128, NH], fp32, kind="ExternalInput").ap()
    bv = nc.dram_tensor("bv", [1, HDc], fp32r, kind="ExternalInput").ap()
    bo = nc.dram_tensor("bo", [1, E], fp32r, kind="ExternalInput").ap()
    ones_d = nc.dram_tensor("ones", [128, 128], fp32r, kind="ExternalInput").ap()
    out = nc.dram_tensor("out", [NH * 128, E], fp32, kind="ExternalOutput").ap()
    # outT spill buffer between attention and O-projection
    ocd = nc.dram_tensor("ocd", [NH, 128, S], fp32r).ap()

    with tile.TileContext(nc) as tc, ExitStack() as ctx:
        consts = ctx.enter_context(tc.tile_pool(name="consts", bufs=1))
        ones_sb = consts.tile([128, 128], fp32r)
        nc.scalar.dma_start(ones_sb[:], ones_d)
        ones_col = ones_sb[:, :1]
        ones_row = ones_sb[:1, :]
        bq_sb = consts.tile([128, NH], fp32)
        bk_sb = consts.tile([128, NH], fp32)
        bv_sb = consts.tile([1, HDc], fp32r)
        nc.scalar.dma_start(bq_sb[:], bq)
        nc.scalar.dma_start(bk_sb[:], bk)
        nc.scalar.dma_start(bv_sb[:], bv)

        with tc.tile_pool(name="persist", bufs=1) as persist, \
             tc.tile_pool(name="xin", bufs=6) as xin:
            qc = persist.tile([128, NH, S], fp32r)    # qT_c: [d, h, s]
            kc = persist.tile([128, NH, S], fp32r)    # kT_c: [d, h, s]
            vc = persist.tile([128, NT, HDc], fp32r)  # v_c: [t_p, t_t, h*128+d]

            # ============== Phase A: q/k/v projections ==============
            with tc.tile_pool(name="wpool", bufs=2) as wpool, \
                 tc.tile_pool(name="pa_psum", bufs=2, space="PSUM") as pa:

                def proj_qk(w_dram, x_dram, bias_sb, dst, engs):
                    # dst[:, m, s*] = W_c @ x^T  (hd x S), bias fused in evict
                    w_sb = wpool.tile([128, NK, HDc], fp32r, tag="w",
                                      name="w_sb")
                    wd = w_dram.rearrange("(k p) c -> k p c", p=128)
                    for k in range(NK):
                        nc.scalar.dma_start(w_sb[:, k, :], wd[k])
                    for s in range(NPC):
                        ps = [pa.tile([128, PCH], fp32, tag=f"pa{m}",
                                      name=f"pa{m}") for m in range(NH)]
                        for k in range(NK):
                            xtile = xin.tile([128, PCH], fp32r, tag="xin")
                            engs[k % len(engs)].dma_start(
                                xtile[:], x_dram[k, s])
                            for m in range(NH):
                                nc.tensor.matmul(
                                    ps[m][:],
                                    w_sb[:, k, m * 128:(m + 1) * 128],
                                    xtile[:], start=(k == 0),
                                    stop=(k == NK - 1))
                        for m in range(NH):
                            nc.vector.tensor_scalar_add(
                                dst[:, m, s * PCH:(s + 1) * PCH], ps[m][:],
                                bias_sb[:, m:m + 1])

                proj_qk(wqT, xqT, bq_sb, qc, [nc.sync, nc.scalar])
                proj_qk(wkT, xkT, bk_sb, kc, [nc.gpsimd, nc.sync])

                def proj_v_group(tc4):
                    # 4 t-tiles of v: stationary = xvT tile slices, rhs = w
                    w_sb = proj_v_group.w_sb
                    ps = [pa.tile([128, HDc], fp32, tag=f"pa{j}",
                                  name=f"pav{j}") for j in range(4)]
                    for k in range(NK):
                        xtile = xin.tile([128, PCH], fp32r, tag="xin")
                        nc.scalar.dma_start(xtile[:], xvT[k, tc4])
                        for j in range(4):
                            nc.tensor.matmul(
                                ps[j][:], xtile[:, j * 128:(j + 1) * 128],
                                w_sb[:, k, :], start=(k == 0), stop=False)
                    for j in range(4):
                        nc.tensor.matmul(ps[j][:], ones_row, bv_sb[:],
                                         start=False, stop=True)
                        nc.vector.tensor_copy(vc[:, tc4 * 4 + j, :], ps[j][:])

                w_sb = wpool.tile([128, NK, HDc], fp32r, tag="w",
                                  name="wv_sb")
                wvd = wvT.rearrange("(k p) c -> k p c", p=128)
                for k in range(NK):
                    nc.scalar.dma_start(w_sb[:, k, :], wvd[k])
                proj_v_group.w_sb = w_sb
                for g in range(NT // 4):
                    proj_v_group(g)

            # ============== Phase B: attention ==============
            with tc.tile_pool(name="expp", bufs=2) as expp, \
                 tc.tile_pool(name="bsc", bufs=2) as bsc, \
                 tc.tile_pool(name="ocsb", bufs=2) as ocsb, \
                 tc.tile_pool(name="st_psum", bufs=4, space="PSUM") as stp:

                def scores_exp(h, c):
                    cs = slice(c * CH, (c + 1) * CH)
                    expT = expp.tile([128, NT, CH], fp32r, tag="expT",
                                     name=f"expT_{h}_{c}")
                    for tt in range(NT):
                        ps = stp.tile([128, CH], fp32, tag="st", name="st")
                        nc.tensor.matmul(
                            ps[:], kc[:, h, tt * 128:(tt + 1) * 128],
                            qc[:, h, cs], start=True, stop=True)
                        nc.scalar.activation(expT[:, tt, :], ps[:],
                                             AF.Exp, scale=inv_sqrt_dk)
                    return expT

                def attn_tail(h, c, expT, otp, dnp):
                    cs = slice(c * CH, (c + 1) * CH)
                    # outT and denominator column-sum interleaved; both are
                    # PSUM-accumulated matmul chains over the 16 t-tiles
                    op = otp.tile([128, CH], fp32, tag="ot", name="ot")
                    dn = dnp.tile([1, CH], fp32, tag="dn", name="dn")
                    for tt in range(NT):
                        nc.tensor.matmul(
                            op[:], vc[:, tt, h * 128:(h + 1) * 128],
                            expT[:, tt, :], start=(tt == 0),
                            stop=(tt == NT - 1))
                        nc.tensor.matmul(
                            dn[:], ones_col, expT[:, tt, :],
                            start=(tt == 0), stop=(tt == NT - 1))
                    dn_sb = bsc.tile([1, CH], fp32r, tag="dnsb", name="dnsb")
                    nc.vector.tensor_copy(dn_sb[:], dn[:])
                    dbc = dnp.tile([128, CH], fp32, tag="dn", name="dbc")
                    nc.tensor.matmul(dbc[:], ones_row, dn_sb[:],
                                     start=True, stop=True)
                    rsc = bsc.tile([128, CH], fp32, tag="rsc", name="rsc")
                    nc.vector.reciprocal_approx_fast(rsc[:], dbc[:])
                    oc_t = ocsb.tile([128, CH], fp32r, tag="oct", name="oct")
                    nc.vector.tensor_mul(oc_t[:], op[:], rsc[:])
                    nc.sync.dma_start(ocd[h, :, cs], oc_t[:])

                pairs = [(h, c) for h in range(NH) for c in range(NCH)]
                with tc.tile_pool(name="ot_psum", bufs=2,
                                  space="PSUM") as otp, \
                     tc.tile_pool(name="dn_psum", bufs=2,
                                  space="PSUM") as dnp:
                    prev = None
                    for h, c in pairs:
                        expT = scores_exp(h, c)
                        if prev is not None:
                            attn_tail(*prev, otp, dnp)
                        prev = (h, c, expT)
                    attn_tail(*prev, otp, dnp)

        # ============== Phase C: output projection ==============
        with tc.tile_pool(name="ocin", bufs=1) as ocin, \
             tc.tile_pool(name="wo_in", bufs=6) as wo_in, \
             tc.tile_pool(name="osb", bufs=8) as osb, \
             tc.tile_pool(name="pc_psum", bufs=2, space="PSUM") as pc:
            bo_sb = ocin.tile([1, E], fp32r, name="bo_sb")
            nc.scalar.dma_start(bo_sb[:], bo)
            oc_h = []
            for h in range(NH):
                t = ocin.tile([128, S], fp32r, name=f"ocin{h}")
                nc.scalar.dma_start(t[:], ocd[h])
                oc_h.append(t)
            wot = woT.rearrange("(k p) e -> k p e", p=128)
            for nn in range(NOC):
                ns = slice(nn * PCH, (nn + 1) * PCH)
                ps = [pc.tile([128, PCH], fp32, tag=f"pc{h}", name=f"pc{h}")
                      for h in range(NH)]
                for k in range(NK):
                    wtile = wo_in.tile([128, PCH], fp32r, tag="wo")
                    eng = nc.sync if k % 2 == 0 else nc.gpsimd
                    eng.dma_start(wtile[:], wot[k, :, ns])
                    for h in range(NH):
                        # lhsT = X_h^T k-tile: strided view of outT
                        lhs = oc_h[h].rearrange(
                            "p (j i) -> p i j", i=NK)[:, k, :]
                        nc.tensor.matmul(ps[h][:], lhs, wtile[:],
                                         start=(k == 0), stop=False)
                for h in range(NH):
                    nc.tensor.matmul(ps[h][:], ones_row, bo_sb[:, ns],
                                     start=False, stop=True)
                    ot = osb.tile([128, PCH], fp32, tag="osb")
                    nc.vector.tensor_copy(ot[:], ps[h][:])
                    nc.sync.dma_start(out[h * 128:(h + 1) * 128, ns], ot[:])

    nc.compile()
    return nc


def _tile_x(xt, NK, NPC, PCH):
    # (E, S) -> [k, s_chunk, partition, col] contiguous
    return np.ascontiguousarray(
        xt.reshape(NK, 128, NPC, PCH).transpose(0, 2, 1, 3))


def shard_inputs(cfg: Cfg, query, key, value, Wq, bq, Wk, bk, Wv, bv, Wo, bo):
    """Build per-core in_maps from full inputs."""
    f = np.float32
    query, key, value = (np.asarray(a, f) for a in (query, key, value))
    Wq, Wk, Wv, Wo = (np.asarray(a, f) for a in (Wq, Wk, Wv, Wo))
    bq, bk, bv, bo = (np.asarray(a, f) for a in (bq, bk, bv, bo))
    NH, HDc, NK, NPC, PCH = cfg.NH, cfg.HDc, cfg.NK, cfg.NPC, cfg.PCH
    woT = np.ascontiguousarray(Wo.T)
    _ONES = np.ones((128, 128), np.float32)
    bo_r = np.ascontiguousarray(bo.reshape(1, -1))
    xq_t = [_tile_x(query[n].T, NK, NPC, PCH) for n in range(N_BATCH)]
    xk_t = [_tile_x(key[n].T, NK, NPC, PCH) for n in range(N_BATCH)]
    xv_t = [_tile_x(value[n].T, NK, NPC, PCH) for n in range(N_BATCH)]
    in_maps = []
    cores_per_batch = N_CORES // N_BATCH
    for c in range(N_CORES):
        n = c // cores_per_batch
        hs = (c % cores_per_batch) * HDc
        sl = slice(hs, hs + HDc)
        in_maps.append({
            "xqT": xq_t[n],
            "xkT": xk_t[n],
            "xvT": xv_t[n],
            "wqT": np.ascontiguousarray(Wq[sl].T),
            "wkT": np.ascontiguousarray(Wk[sl].T),
            "wvT": np.ascontiguousarray(Wv[sl].T),
            "woT": woT,
            "bq": np.ascontiguousarray(bq[sl].reshape(NH, 128).T),
            "bk": np.ascontiguousarray(bk[sl].reshape(NH, 128).T),
            "bv": np.ascontiguousarray(bv[sl].reshape(1, HDc)),
            "bo": bo_r,
            "ones": _ONES,
        })
    return in_maps


def gather_outputs(cfg: Cfg, results):
    """results: list of per-core {'out': (NH*128, E)} -> full (N, S, E)."""
    E = cfg.E
    full = np.empty((N_BATCH, SEQ, E), np.float32)
    cores_per_batch = N_CORES // N_BATCH
    rows = cfg.NH * 128
    for c in range(N_CORES):
        n = c // cores_per_batch
        r0 = (c % cores_per_batch) * rows
        full[n, r0:r0 + rows, :] = results[c]["out"]
    return full


_CACHE = {}


def kernel(**inputs) -> np.ndarray:
    from concourse.bass_utils import run_bass_kernel_spmd
    cfg = Cfg()
    if "nc" not in _CACHE:
        _CACHE["nc"] = build_program(cfg)
    nc = _CACHE["nc"]
    in_maps = shard_inputs(cfg, **inputs)
    res = run_bass_kernel_spmd(nc, in_maps, core_ids=list(range(N_CORES)))
    return gather_outputs(cfg, res.results)


# revision 22
# speedup vs baseline: 1.2030x; 1.0229x over previous
# Multi-head attention (N=2, S=2048, E=2048, H=16, Dk=128) on 8 NeuronCores.
#
# Sharding: 2 batches x 16 heads = 32 (n,h) pairs -> core c owns batch c//4,
# heads (c%4)*4 .. +4. The reference reshapes (N,H,S,Dk)->(N,S,H*Dk) without
# a head transpose, so rows [h*128,(h+1)*128) of the pre-projection matrix X
# (and hence of the final output) depend on head h only: each core computes
# 512 disjoint output rows and the host concatenates. No collectives.
#
# Device math per core (all matmuls fp32r, transposed layouts):
#   qT_c = Wq_c @ query[n].T   (hd x S)   kT_c same     v_c = value[n] @ Wv_c.T (S x hd)
#   sT   = k_h^T-tiles @ qT_h  (t x s)    expT = exp(sT/sqrt(Dk))
#   outT = v_h-tiles.T @ expT  (d x s)    denom: in-place add-tree + ones-matmul
#   out  = X_h @ Wo.T + bo  with X_h^T k-tiles = strided views of outT
#
# Perf notes (v3):
#  - all matmuls N=512 so the per-matmul fp32 weight load hides under the
#    ~225ns stream; x inputs are host-pretiled so every DMA reads a
#    contiguous 256KB block; weight DMAs are per-k so the first matmul
#    only waits for one tile.
#  - DMA issue (~0.6-0.8us sequencer time each) is spread across the
#    sync/gpsimd/scalar/vector queues; nothing is DMA-triggered from the
#    Scalar queue while it runs exps.
#  - v-projection is interleaved with the first two head-0 score chunks
#    so the Scalar engine (exp) starts early; exp runs on 2-bank psum
#    pairs (one (128,1024) op per two score tiles).
#  - softmax denominator: in-place add-tree on GpSimd (idle otherwise)
#    + ones-matmul column sum + PE partition-broadcast + fast reciprocal.
#  - outT spills to DRAM between phases to stay under the SBUF cap.
import numpy as np

D_MODEL = 2048
NHEAD = 16
DK = 128
N_BATCH = 2
SEQ = 2048
N_CORES = 8
HEADS_PER_CORE = 4


class Cfg:
    def __init__(self, S=SEQ, E=D_MODEL, NH=HEADS_PER_CORE, CH=512):
        assert S % 128 == 0 and E % 128 == 0
        self.S = S          # sequence length
        self.E = E          # model dim (contraction for projections)
        self.NH = NH        # heads per core
        self.CH = CH        # s-chunk width for attention phase
        self.NK = E // 128  # contraction tiles for projections / O-proj
        self.NT = S // 128  # t tiles (attention contraction)
        self.HDc = NH * DK  # head dims per core
        self.RPH = (S * DK) // E  # output rows per head (=128 at full size)
        assert self.RPH == 128, "O-proj layout assumes 128 rows per head"
        self.NCH = S // CH  # number of s-chunks
        assert S % CH == 0 and CH >= 256  # fp32r full-rate needs N>=256
        self.PCH = 512      # projection / O-proj free-dim chunk
        self.NPC = S // self.PCH   # projection s-chunks
        self.NOC = E // self.PCH   # O-proj output chunks


def build_program(cfg: Cfg):
    import concourse.bass as bass
    import concourse.tile as tile
    from concourse import bacc, mybir
    from contextlib import ExitStack

    fp32 = mybir.dt.float32
    fp32r = mybir.dt.float32r
    AF = mybir.ActivationFunctionType

    S, E, NH, CH = cfg.S, cfg.E, cfg.NH, cfg.CH
    NK, NT, HDc = cfg.NK, cfg.NT, cfg.HDc
    PCH, NPC, NOC, NCH = cfg.PCH, cfg.NPC, cfg.NOC, cfg.NCH
    inv_sqrt_dk = 1.0 / float(np.sqrt(DK))

    nc = bacc.Bacc("TRN2", target_bir_lowering=False, debug=False,
                   num_devices=N_CORES)

    # DRAM I/O (per-core values supplied via in_maps).
    # x inputs are host-pretiled: [k, s_chunk, partition, col].
    xqT = nc.dram_tensor("xqT", [NK, NPC, 128, PCH], fp32r,
                         kind="ExternalInput").ap()
    xkT = nc.dram_tensor("xkT", [NK, NPC, 128, PCH], fp32r,
                         kind="ExternalInput").ap()
    xvT = nc.dram_tensor("xvT", [NK, NPC, 128, PCH], fp32r,
                         kind="ExternalInput").ap()
    wqT = nc.dram_tensor("wqT", [E, HDc], fp32r, kind="ExternalInput").ap()
    wkT = nc.dram_tensor("wkT", [E, HDc], fp32r, kind="ExternalInput").ap()
    wvT = nc.dram_tensor("wvT", [E, HDc], fp32r, kind="ExternalInput").ap()
    woT = nc.dram_tensor("woT", [E, E], fp32r, kind="ExternalInput").ap()
    bq = nc.dram_tensor("bq", [128, NH], fp32, kind="ExternalInput").ap()
    bk = nc.dram_tensor("bk", [128, NH], fp32, kind="ExternalInput").ap()
    bv = nc.dram_tensor("bv", [1, HDc], fp32r, kind="ExternalInput").ap()
    bo = nc.dram_tensor("bo", [1, E], fp32r, kind="ExternalInput").ap()
    ones_d = nc.dram_tensor("ones", [128, 128], fp32r, kind="ExternalInput").ap()
    out = nc.dram_tensor("out", [NH * 128, E], fp32, kind="ExternalOutput").ap()
    # outT spill buffer between attention and O-projection
    ocd = nc.dram_tensor("ocd", [NH, 128, S], fp32r).ap()

    with tile.TileContext(nc) as tc, ExitStack() as ctx:
        consts = ctx.enter_context(tc.tile_pool(name="consts", bufs=1))
        wq_pre = consts.tile([128, 4, HDc], fp32r, name="wq_pre")
        wqd = wqT.rearrange("(k p) c -> k p c", p=128)
        for k in range(4):
            nc.scalar.dma_start(wq_pre[:, k, :], wqd[k])
        ones_sb = consts.tile([128, 128], fp32r)
        nc.scalar.dma_start(ones_sb[:], ones_d)
        ones_col = ones_sb[:, :1]
        ones_row = ones_sb[:1, :]
        bq_sb = consts.tile([128, NH], fp32)
        bk_sb = consts.tile([128, NH], fp32)
        bv_sb = consts.tile([1, HDc], fp32r)
        nc.scalar.dma_start(bq_sb[:], bq)
        nc.scalar.dma_start(bk_sb[:], bk)
        nc.scalar.dma_start(bv_sb[:], bv)

        with tc.tile_pool(name="persist", bufs=1) as persist, \
             tc.tile_pool(name="xin", bufs=6) as xin:
            qc = persist.tile([128, NH, S], fp32r)    # qT_c: [d, h, s]
            kc = persist.tile([128, NH, S], fp32r)    # kT_c: [d, h, s]
            vc = persist.tile([128, NT, HDc], fp32r)  # v_c: [t_p, t_t, h*128+d]

            # ============== Phase A: q/k/v projections ==============
            with tc.tile_pool(name="wpool", bufs=2) as wpool, \
                 tc.tile_pool(name="pa_psum", bufs=2, space="PSUM") as pa:

                def proj_qk(w_dram, x_dram, bias_sb, dst, engs,
                            pre=None):
                    # dst[:, m, s*] = W_c @ x^T  (hd x S), bias fused in evict
                    w_sb = wpool.tile([128, NK, HDc], fp32r, tag="w",
                                      name="w_sb")
                    wd = w_dram.rearrange("(k p) c -> k p c", p=128)
                    npre = 0 if pre is None else pre.shape[1]
                    for k in range(npre, NK):
                        nc.scalar.dma_start(w_sb[:, k, :], wd[k])

                    def wslice(k, m):
                        if k < npre:
                            return pre[:, k, m * 128:(m + 1) * 128]
                        return w_sb[:, k, m * 128:(m + 1) * 128]
                    for s in range(NPC):
                        ps = [pa.tile([128, PCH], fp32, tag=f"pa{m}",
                                      name=f"pa{m}") for m in range(NH)]
                        for k in range(NK):
                            xtile = xin.tile([128, PCH], fp32r, tag="xin")
                            engs[k % len(engs)].dma_start(
                                xtile[:], x_dram[k, s])
                            for m in range(NH):
                                nc.tensor.matmul(
                                    ps[m][:], wslice(k, m),
                                    xtile[:], start=(k == 0),
                                    stop=(k == NK - 1))
                        for m in range(NH):
                            nc.vector.tensor_scalar_add(
                                dst[:, m, s * PCH:(s + 1) * PCH], ps[m][:],
                                bias_sb[:, m:m + 1])

                proj_qk(wqT, xqT, bq_sb, qc, [nc.sync, nc.scalar],
                        pre=wq_pre)
                proj_qk(wkT, xkT, bk_sb, kc, [nc.gpsimd, nc.sync])

                def proj_v_group(tc4):
                    # 4 t-tiles of v: stationary = xvT tile slices, rhs = w
                    w_sb = proj_v_group.w_sb
                    ps = [pa.tile([128, HDc], fp32, tag=f"pa{j}",
                                  name=f"pav{j}") for j in range(4)]
                    for k in range(NK):
                        xtile = xin.tile([128, PCH], fp32r, tag="xin")
                        nc.scalar.dma_start(xtile[:], xvT[k, tc4])
                        for j in range(4):
                            nc.tensor.matmul(
                                ps[j][:], xtile[:, j * 128:(j + 1) * 128],
                                w_sb[:, k, :], start=(k == 0), stop=False)
                    for j in range(4):
                        nc.tensor.matmul(ps[j][:], ones_row, bv_sb[:],
                                         start=False, stop=True)
                        nc.vector.tensor_copy(vc[:, tc4 * 4 + j, :], ps[j][:])

                w_sb = wpool.tile([128, NK, HDc], fp32r, tag="w",
                                  name="wv_sb")
                wvd = wvT.rearrange("(k p) c -> k p c", p=128)
                for k in range(NK):
                    nc.scalar.dma_start(w_sb[:, k, :], wvd[k])
                proj_v_group.w_sb = w_sb
                for g in range(NT // 4):
                    proj_v_group(g)

            # ============== Phase B: attention ==============
            with tc.tile_pool(name="expp", bufs=2) as expp, \
                 tc.tile_pool(name="bsc", bufs=2) as bsc, \
                 tc.tile_pool(name="ocsb", bufs=2) as ocsb, \
                 tc.tile_pool(name="st_psum", bufs=4, space="PSUM") as stp:

                def scores_exp(h, c):
                    cs = slice(c * CH, (c + 1) * CH)
                    expT = expp.tile([128, NT, CH], fp32r, tag="expT",
                                     name=f"expT_{h}_{c}")
                    for tt in range(NT):
                        ps = stp.tile([128, CH], fp32, tag="st", name="st")
                        nc.tensor.matmul(
                            ps[:], kc[:, h, tt * 128:(tt + 1) * 128],
                            qc[:, h, cs], start=True, stop=True)
                        nc.scalar.activation(expT[:, tt, :], ps[:],
                                             AF.Exp, scale=inv_sqrt_dk)
                    return expT

                def attn_tail(h, c, expT, otp, dnp):
                    cs = slice(c * CH, (c + 1) * CH)
                    # outT and denominator column-sum interleaved; both are
                    # PSUM-accumulated matmul chains over the 16 t-tiles
                    op = otp.tile([128, CH], fp32, tag="ot", name="ot")
                    dn = dnp.tile([1, CH], fp32, tag="dn", name="dn")
                    for tt in range(NT):
                        nc.tensor.matmul(
                            op[:], vc[:, tt, h * 128:(h + 1) * 128],
                            expT[:, tt, :], start=(tt == 0),
                            stop=(tt == NT - 1))
                        nc.tensor.matmul(
                            dn[:], ones_col, expT[:, tt, :],
                            start=(tt == 0), stop=(tt == NT - 1))
                    dn_sb = bsc.tile([1, CH], fp32r, tag="dnsb", name="dnsb")
                    nc.vector.tensor_copy(dn_sb[:], dn[:])
                    dbc = dnp.tile([128, CH], fp32, tag="dn", name="dbc")
                    nc.tensor.matmul(dbc[:], ones_row, dn_sb[:],
                                     start=True, stop=True)
                    rsc = bsc.tile([128, CH], fp32, tag="rsc", name="rsc")
                    nc.vector.reciprocal_approx_fast(rsc[:], dbc[:])
                    oc_t = ocsb.tile([128, CH], fp32r, tag="oct", name="oct")
                    nc.vector.tensor_mul(oc_t[:], op[:], rsc[:])
                    nc.sync.dma_start(ocd[h, :, cs], oc_t[:])

                pairs = [(h, c) for h in range(NH) for c in range(NCH)]
                with tc.tile_pool(name="ot_psum", bufs=2,
                                  space="PSUM") as otp, \
                     tc.tile_pool(name="dn_psum", bufs=2,
                                  space="PSUM") as dnp:
                    prev = None
                    for h, c in pairs:
                        expT = scores_exp(h, c)
                        if prev is not None:
                            attn_tail(*prev, otp, dnp)
                        prev = (h, c, expT)
                    attn_tail(*prev, otp, dnp)

        # ============== Phase C: output projection ==============
        with tc.tile_pool(name="ocin", bufs=1) as ocin, \
             tc.tile_pool(name="wo_in", bufs=6) as wo_in, \
             tc.tile_pool(name="osb", bufs=8) as osb, \
             tc.tile_pool(name="pc_psum", bufs=2, space="PSUM") as pc:
            wot = woT.rearrange("(k p) e -> k p e", p=128)
            w0 = wo_in.tile([128, PCH], fp32r, tag="wo", name="wo0")
            nc.sync.dma_start(w0[:], wot[0, :, 0:PCH])
            bo_sb = ocin.tile([1, E], fp32r, name="bo_sb")
            nc.scalar.dma_start(bo_sb[:], bo)
            oc_h = []
            engs3 = [nc.scalar, nc.gpsimd, nc.sync]
            for h in range(NH):
                t = ocin.tile([128, S], fp32r, name=f"ocin{h}")
                half = S // 2
                engs3[h % 3].dma_start(t[:, :half], ocd[h, :, :half])
                engs3[(h + 1) % 3].dma_start(t[:, half:], ocd[h, :, half:])
                oc_h.append(t)
            for nn in range(NOC):
                ns = slice(nn * PCH, (nn + 1) * PCH)
                ps = [pc.tile([128, PCH], fp32, tag=f"pc{h}", name=f"pc{h}")
                      for h in range(NH)]
                for k in range(NK):
                    if nn == 0 and k == 0:
                        wtile = w0
                    else:
                        wtile = wo_in.tile([128, PCH], fp32r, tag="wo")
                        eng = nc.sync if k % 2 == 0 else nc.gpsimd
                        eng.dma_start(wtile[:], wot[k, :, ns])
                    for h in range(NH):
                        # lhsT = X_h^T k-tile: strided view of outT
                        lhs = oc_h[h].rearrange(
                            "p (j i) -> p i j", i=NK)[:, k, :]
                        nc.tensor.matmul(ps[h][:], lhs, wtile[:],
                                         start=(k == 0), stop=False)
                for h in range(NH):
                    nc.tensor.matmul(ps[h][:], ones_row, bo_sb[:, ns],
                                     start=False, stop=True)
                    ot = osb.tile([128, PCH], fp32, tag="osb")
                    nc.vector.tensor_copy(ot[:], ps[h][:])
                    nc.sync.dma_start(out[h * 128:(h + 1) * 128, ns], ot[:])

    nc.compile()
    return nc


def _tile_x(xt, NK, NPC, PCH):
    # (E, S) -> [k, s_chunk, partition, col] contiguous
    return np.ascontiguousarray(
        xt.reshape(NK, 128, NPC, PCH).transpose(0, 2, 1, 3))


def shard_inputs(cfg: Cfg, query, key, value, Wq, bq, Wk, bk, Wv, bv, Wo, bo):
    """Build per-core in_maps from full inputs."""
    f = np.float32
    query, key, value = (np.asarray(a, f) for a in (query, key, value))
    Wq, Wk, Wv, Wo = (np.asarray(a, f) for a in (Wq, Wk, Wv, Wo))
    bq, bk, bv, bo = (np.asarray(a, f) for a in (bq, bk, bv, bo))
    NH, HDc, NK, NPC, PCH = cfg.NH, cfg.HDc, cfg.NK, cfg.NPC, cfg.PCH
    woT = np.ascontiguousarray(Wo.T)
    _ONES = np.ones((128, 128), np.float32)
    bo_r = np.ascontiguousarray(bo.reshape(1, -1))
    xq_t = [_tile_x(query[n].T, NK, NPC, PCH) for n in range(N_BATCH)]
    xk_t = [_tile_x(key[n].T, NK, NPC, PCH) for n in range(N_BATCH)]
    xv_t = [_tile_x(value[n].T, NK, NPC, PCH) for n in range(N_BATCH)]
    in_maps = []
    cores_per_batch = N_CORES // N_BATCH
    for c in range(N_CORES):
        n = c // cores_per_batch
        hs = (c % cores_per_batch) * HDc
        sl = slice(hs, hs + HDc)
        in_maps.append({
            "xqT": xq_t[n],
            "xkT": xk_t[n],
            "xvT": xv_t[n],
            "wqT": np.ascontiguousarray(Wq[sl].T),
            "wkT": np.ascontiguousarray(Wk[sl].T),
            "wvT": np.ascontiguousarray(Wv[sl].T),
            "woT": woT,
            "bq": np.ascontiguousarray(bq[sl].reshape(NH, 128).T),
            "bk": np.ascontiguousarray(bk[sl].reshape(NH, 128).T),
            "bv": np.ascontiguousarray(bv[sl].reshape(1, HDc)),
            "bo": bo_r,
            "ones": _ONES,
        })
    return in_maps


def gather_outputs(cfg: Cfg, results):
    """results: list of per-core {'out': (NH*128, E)} -> full (N, S, E)."""
    E = cfg.E
    full = np.empty((N_BATCH, SEQ, E), np.float32)
    cores_per_batch = N_CORES // N_BATCH
    rows = cfg.NH * 128
    for c in range(N_CORES):
        n = c // cores_per_batch
        r0 = (c % cores_per_batch) * rows
        full[n, r0:r0 + rows, :] = results[c]["out"]
    return full


_CACHE = {}


def kernel(**inputs) -> np.ndarray:
    from concourse.bass_utils import run_bass_kernel_spmd
    cfg = Cfg()
    if "nc" not in _CACHE:
        _CACHE["nc"] = build_program(cfg)
    nc = _CACHE["nc"]
    in_maps = shard_inputs(cfg, **inputs)
    res = run_bass_kernel_spmd(nc, in_maps, core_ids=list(range(N_CORES)))
    return gather_outputs(cfg, res.results)


# revision 23
# speedup vs baseline: 1.2581x; 1.0458x over previous
# Multi-head attention (N=2, S=2048, E=2048, H=16, Dk=128) on 8 NeuronCores.
#
# Sharding: 2 batches x 16 heads = 32 (n,h) pairs -> core c owns batch c//4,
# heads (c%4)*4 .. +4. The reference reshapes (N,H,S,Dk)->(N,S,H*Dk) without
# a head transpose, so rows [h*128,(h+1)*128) of the pre-projection matrix X
# (and hence of the final output) depend on head h only: each core computes
# 512 disjoint output rows and the host concatenates. No collectives.
#
# Device math per core (all matmuls fp32r, transposed layouts):
#   qT_c = Wq_c @ query[n].T   (hd x S)   kT_c same     v_c = value[n] @ Wv_c.T (S x hd)
#   sT   = k_h^T-tiles @ qT_h  (t x s)    expT = exp(sT/sqrt(Dk))
#   outT = v_h-tiles.T @ expT  (d x s)    denom: in-place add-tree + ones-matmul
#   out  = X_h @ Wo.T + bo  with X_h^T k-tiles = strided views of outT
#
# Perf notes (v3):
#  - all matmuls N=512 so the per-matmul fp32 weight load hides under the
#    ~225ns stream; x inputs are host-pretiled so every DMA reads a
#    contiguous 256KB block; weight DMAs are per-k so the first matmul
#    only waits for one tile.
#  - DMA issue (~0.6-0.8us sequencer time each) is spread across the
#    sync/gpsimd/scalar/vector queues; nothing is DMA-triggered from the
#    Scalar queue while it runs exps.
#  - v-projection is interleaved with the first two head-0 score chunks
#    so the Scalar engine (exp) starts early; exp runs on 2-bank psum
#    pairs (one (128,1024) op per two score tiles).
#  - softmax denominator: in-place add-tree on GpSimd (idle otherwise)
#    + ones-matmul column sum + PE partition-broadcast + fast reciprocal.
#  - outT spills to DRAM between phases to stay under the SBUF cap.
import numpy as np

D_MODEL = 2048
NHEAD = 16
DK = 128
N_BATCH = 2
SEQ = 2048
N_CORES = 8
HEADS_PER_CORE = 4


class Cfg:
    def __init__(self, S=SEQ, E=D_MODEL, NH=HEADS_PER_CORE, CH=512):
        assert S % 128 == 0 and E % 128 == 0
        self.S = S          # sequence length
        self.E = E          # model dim (contraction for projections)
        self.NH = NH        # heads per core
        self.CH = CH        # s-chunk width for attention phase
        self.NK = E // 128  # contraction tiles for projections / O-proj
        self.NT = S // 128  # t tiles (attention contraction)
        self.HDc = NH * DK  # head dims per core
        self.RPH = (S * DK) // E  # output rows per head (=128 at full size)
        assert self.RPH == 128, "O-proj layout assumes 128 rows per head"
        self.NCH = S // CH  # number of s-chunks
        assert S % CH == 0 and CH >= 256  # fp32r full-rate needs N>=256
        self.PCH = 512      # projection / O-proj free-dim chunk
        self.NPC = S // self.PCH   # projection s-chunks
        self.NOC = E // self.PCH   # O-proj output chunks


def build_program(cfg: Cfg):
    import concourse.bass as bass
    import concourse.tile as tile
    from concourse import bacc, mybir
    from contextlib import ExitStack

    fp32 = mybir.dt.float32
    fp32r = mybir.dt.float32r
    AF = mybir.ActivationFunctionType

    S, E, NH, CH = cfg.S, cfg.E, cfg.NH, cfg.CH
    NK, NT, HDc = cfg.NK, cfg.NT, cfg.HDc
    PCH, NPC, NOC, NCH = cfg.PCH, cfg.NPC, cfg.NOC, cfg.NCH
    inv_sqrt_dk = 1.0 / float(np.sqrt(DK))

    nc = bacc.Bacc("TRN2", target_bir_lowering=False, debug=False,
                   num_devices=N_CORES)

    # DRAM I/O (per-core values supplied via in_maps).
    # x inputs are host-pretiled: [k, s_chunk, partition, col].
    xqT = nc.dram_tensor("xqT", [NK, NPC, 128, PCH], fp32r,
                         kind="ExternalInput").ap()
    xkT = nc.dram_tensor("xkT", [NK, NPC, 128, PCH], fp32r,
                         kind="ExternalInput").ap()
    xvT = nc.dram_tensor("xvT", [NK, NPC, 128, PCH], fp32r,
                         kind="ExternalInput").ap()
    wqT = nc.dram_tensor("wqT", [E, HDc], fp32r, kind="ExternalInput").ap()
    wkT = nc.dram_tensor("wkT", [E, HDc], fp32r, kind="ExternalInput").ap()
    wvT = nc.dram_tensor("wvT", [E, HDc], fp32r, kind="ExternalInput").ap()
    woT = nc.dram_tensor("woT", [E, E], fp32r, kind="ExternalInput").ap()
    bq = nc.dram_tensor("bq", [128, NH], fp32, kind="ExternalInput").ap()
    bk = nc.dram_tensor("bk", [128, NH], fp32, kind="ExternalInput").ap()
    bv = nc.dram_tensor("bv", [1, HDc], fp32r, kind="ExternalInput").ap()
    bo = nc.dram_tensor("bo", [1, E], fp32r, kind="ExternalInput").ap()
    ones_d = nc.dram_tensor("ones", [128, 128], fp32r, kind="ExternalInput").ap()
    out = nc.dram_tensor("out", [NH * 128, E], fp32, kind="ExternalOutput").ap()
    # outT spill buffer between attention and O-projection
    ocd = nc.dram_tensor("ocd", [NH, 128, S], fp32r).ap()

    with tile.TileContext(nc) as tc, ExitStack() as ctx:
        consts = ctx.enter_context(tc.tile_pool(name="consts", bufs=1))
        wq_pre = consts.tile([128, 4, HDc], fp32r, name="wq_pre")
        wqd = wqT.rearrange("(k p) c -> k p c", p=128)
        for k in range(4):
            nc.scalar.dma_start(wq_pre[:, k, :], wqd[k])
        ones_sb = consts.tile([128, 128], fp32r)
        ones_col = ones_sb[:, :1]
        ones_row = ones_sb[:1, :]
        bq_sb = consts.tile([128, NH], fp32)
        bk_sb = consts.tile([128, NH], fp32)
        bv_sb = consts.tile([1, HDc], fp32r)

        def emit_const_loads():
            nc.scalar.dma_start(ones_sb[:], ones_d)
            nc.scalar.dma_start(bq_sb[:], bq)
            nc.scalar.dma_start(bk_sb[:], bk)
            nc.scalar.dma_start(bv_sb[:], bv)

        with tc.tile_pool(name="persist", bufs=1) as persist, \
             tc.tile_pool(name="xin", bufs=6) as xin:
            qc = persist.tile([128, NH, S], fp32r)    # qT_c: [d, h, s]
            kc = persist.tile([128, NH, S], fp32r)    # kT_c: [d, h, s]
            vc = persist.tile([128, NT, HDc], fp32r)  # v_c: [t_p, t_t, h*128+d]

            # ============== Phase A: q/k/v projections ==============
            with tc.tile_pool(name="wpool", bufs=2) as wpool, \
                 tc.tile_pool(name="pa_psum", bufs=2, space="PSUM") as pa:

                def proj_qk(w_dram, x_dram, bias_sb, dst, engs,
                            pre=None):
                    # dst[:, m, s*] = W_c @ x^T  (hd x S), bias fused in evict
                    w_sb = wpool.tile([128, NK, HDc], fp32r, tag="w",
                                      name="w_sb")
                    wd = w_dram.rearrange("(k p) c -> k p c", p=128)
                    npre = 0 if pre is None else pre.shape[1]
                    for k in range(npre, NK):
                        nc.scalar.dma_start(w_sb[:, k, :], wd[k])
                    if pre is not None:
                        emit_const_loads()

                    def wslice(k, m):
                        if k < npre:
                            return pre[:, k, m * 128:(m + 1) * 128]
                        return w_sb[:, k, m * 128:(m + 1) * 128]
                    for s in range(NPC):
                        ps = [pa.tile([128, PCH], fp32, tag=f"pa{m}",
                                      name=f"pa{m}") for m in range(NH)]
                        for k in range(NK):
                            xtile = xin.tile([128, PCH], fp32r, tag="xin")
                            engs[k % len(engs)].dma_start(
                                xtile[:], x_dram[k, s])
                            for m in range(NH):
                                nc.tensor.matmul(
                                    ps[m][:], wslice(k, m),
                                    xtile[:], start=(k == 0),
                                    stop=(k == NK - 1))
                        for m in range(NH):
                            nc.vector.tensor_scalar_add(
                                dst[:, m, s * PCH:(s + 1) * PCH], ps[m][:],
                                bias_sb[:, m:m + 1])

                proj_qk(wqT, xqT, bq_sb, qc, [nc.sync],
                        pre=wq_pre)
                proj_qk(wkT, xkT, bk_sb, kc, [nc.gpsimd])

                def proj_v_group(tc4):
                    # 4 t-tiles of v: stationary = xvT tile slices, rhs = w
                    w_sb = proj_v_group.w_sb
                    ps = [pa.tile([128, HDc], fp32, tag=f"pa{j}",
                                  name=f"pav{j}") for j in range(4)]
                    for k in range(NK):
                        xtile = xin.tile([128, PCH], fp32r, tag="xin")
                        nc.scalar.dma_start(xtile[:], xvT[k, tc4])
                        for j in range(4):
                            nc.tensor.matmul(
                                ps[j][:], xtile[:, j * 128:(j + 1) * 128],
                                w_sb[:, k, :], start=(k == 0), stop=False)
                    for j in range(4):
                        nc.tensor.matmul(ps[j][:], ones_row, bv_sb[:],
                                         start=False, stop=True)
                        nc.vector.tensor_copy(vc[:, tc4 * 4 + j, :], ps[j][:])

                w_sb = wpool.tile([128, NK, HDc], fp32r, tag="w",
                                  name="wv_sb")
                wvd = wvT.rearrange("(k p) c -> k p c", p=128)
                for k in range(NK):
                    nc.scalar.dma_start(w_sb[:, k, :], wvd[k])
                proj_v_group.w_sb = w_sb
                for g in range(NT // 4):
                    proj_v_group(g)

            # ============== Phase B: attention ==============
            with tc.tile_pool(name="expp", bufs=2) as expp, \
                 tc.tile_pool(name="bsc", bufs=2) as bsc, \
                 tc.tile_pool(name="ocsb", bufs=2) as ocsb, \
                 tc.tile_pool(name="st_psum", bufs=4, space="PSUM") as stp:

                def scores_exp(h, c):
                    cs = slice(c * CH, (c + 1) * CH)
                    expT = expp.tile([128, NT, CH], fp32r, tag="expT",
                                     name=f"expT_{h}_{c}")
                    for tt in range(NT):
                        ps = stp.tile([128, CH], fp32, tag="st", name="st")
                        nc.tensor.matmul(
                            ps[:], kc[:, h, tt * 128:(tt + 1) * 128],
                            qc[:, h, cs], start=True, stop=True)
                        nc.scalar.activation(expT[:, tt, :], ps[:],
                                             AF.Exp, scale=inv_sqrt_dk)
                    return expT

                def attn_tail(h, c, expT, otp, dnp):
                    cs = slice(c * CH, (c + 1) * CH)
                    # outT and denominator column-sum interleaved; both are
                    # PSUM-accumulated matmul chains over the 16 t-tiles
                    op = otp.tile([128, CH], fp32, tag="ot", name="ot")
                    dn = dnp.tile([1, CH], fp32, tag="dn", name="dn")
                    for tt in range(NT):
                        nc.tensor.matmul(
                            op[:], vc[:, tt, h * 128:(h + 1) * 128],
                            expT[:, tt, :], start=(tt == 0),
                            stop=(tt == NT - 1))
                        nc.tensor.matmul(
                            dn[:], ones_col, expT[:, tt, :],
                            start=(tt == 0), stop=(tt == NT - 1))
                    dn_sb = bsc.tile([1, CH], fp32r, tag="dnsb", name="dnsb")
                    nc.vector.tensor_copy(dn_sb[:], dn[:])
                    dbc = dnp.tile([128, CH], fp32, tag="dn", name="dbc")
                    nc.tensor.matmul(dbc[:], ones_row, dn_sb[:],
                                     start=True, stop=True)
                    rsc = bsc.tile([128, CH], fp32, tag="rsc", name="rsc")
                    nc.vector.reciprocal_approx_fast(rsc[:], dbc[:])
                    oc_t = ocsb.tile([128, CH], fp32r, tag="oct", name="oct")
                    nc.vector.tensor_mul(oc_t[:], op[:], rsc[:])
                    nc.sync.dma_start(ocd[h, :, cs], oc_t[:])

                pairs = [(h, c) for h in range(NH) for c in range(NCH)]
                with tc.tile_pool(name="ot_psum", bufs=2,
                                  space="PSUM") as otp, \
                     tc.tile_pool(name="dn_psum", bufs=2,
                                  space="PSUM") as dnp:
                    prev = None
                    for h, c in pairs:
                        expT = scores_exp(h, c)
                        if prev is not None:
                            attn_tail(*prev, otp, dnp)
                        prev = (h, c, expT)
                    attn_tail(*prev, otp, dnp)

        # ============== Phase C: output projection ==============
        with tc.tile_pool(name="ocin", bufs=1) as ocin, \
             tc.tile_pool(name="wo_in", bufs=6) as wo_in, \
             tc.tile_pool(name="osb", bufs=8) as osb, \
             tc.tile_pool(name="pc_psum", bufs=2, space="PSUM") as pc:
            wot = woT.rearrange("(k p) e -> k p e", p=128)
            w0 = wo_in.tile([128, PCH], fp32r, tag="wo", name="wo0")
            nc.sync.dma_start(w0[:], wot[0, :, 0:PCH])
            bo_sb = ocin.tile([1, E], fp32r, name="bo_sb")
            nc.scalar.dma_start(bo_sb[:], bo)
            oc_h = []
            engs3 = [nc.scalar, nc.gpsimd, nc.sync]
            for h in range(NH):
                t = ocin.tile([128, S], fp32r, name=f"ocin{h}")
                half = S // 2
                engs3[h % 3].dma_start(t[:, :half], ocd[h, :, :half])
                engs3[(h + 1) % 3].dma_start(t[:, half:], ocd[h, :, half:])
                oc_h.append(t)
            for nn in range(NOC):
                ns = slice(nn * PCH, (nn + 1) * PCH)
                ps = [pc.tile([128, PCH], fp32, tag=f"pc{h}", name=f"pc{h}")
                      for h in range(NH)]
                for h in range(NH):
                    nc.tensor.matmul(ps[h][:], ones_row, bo_sb[:, ns],
                                     start=True, stop=False)
                for k in range(NK):
                    if nn == 0 and k == 0:
                        wtile = w0
                    else:
                        wtile = wo_in.tile([128, PCH], fp32r, tag="wo")
                        eng = nc.sync if k % 2 == 0 else nc.gpsimd
                        eng.dma_start(wtile[:], wot[k, :, ns])
                    for h in range(NH):
                        # lhsT = X_h^T k-tile: strided view of outT
                        lhs = oc_h[h].rearrange(
                            "p (j i) -> p i j", i=NK)[:, k, :]
                        nc.tensor.matmul(ps[h][:], lhs, wtile[:],
                                         start=False, stop=(k == NK - 1))
                for h in range(NH):
                    ot = osb.tile([128, PCH], fp32, tag="osb")
                    nc.vector.tensor_copy(ot[:], ps[h][:])
                    nc.sync.dma_start(out[h * 128:(h + 1) * 128, ns], ot[:])

    nc.compile()
    return nc


def _tile_x(xt, NK, NPC, PCH):
    # (E, S) -> [k, s_chunk, partition, col] contiguous
    return np.ascontiguousarray(
        xt.reshape(NK, 128, NPC, PCH).transpose(0, 2, 1, 3))


def shard_inputs(cfg: Cfg, query, key, value, Wq, bq, Wk, bk, Wv, bv, Wo, bo):
    """Build per-core in_maps from full inputs."""
    f = np.float32
    query, key, value = (np.asarray(a, f) for a in (query, key, value))
    Wq, Wk, Wv, Wo = (np.asarray(a, f) for a in (Wq, Wk, Wv, Wo))
    bq, bk, bv, bo = (np.asarray(a, f) for a in (bq, bk, bv, bo))
    NH, HDc, NK, NPC, PCH = cfg.NH, cfg.HDc, cfg.NK, cfg.NPC, cfg.PCH
    woT = np.ascontiguousarray(Wo.T)
    _ONES = np.ones((128, 128), np.float32)
    bo_r = np.ascontiguousarray(bo.reshape(1, -1))
    xq_t = [_tile_x(query[n].T, NK, NPC, PCH) for n in range(N_BATCH)]
    xk_t = [_tile_x(key[n].T, NK, NPC, PCH) for n in range(N_BATCH)]
    xv_t = [_tile_x(value[n].T, NK, NPC, PCH) for n in range(N_BATCH)]
    in_maps = []
    cores_per_batch = N_CORES // N_BATCH
    for c in range(N_CORES):
        n = c // cores_per_batch
        hs = (c % cores_per_batch) * HDc
        sl = slice(hs, hs + HDc)
        in_maps.append({
            "xqT": xq_t[n],
            "xkT": xk_t[n],
            "xvT": xv_t[n],
            "wqT": np.ascontiguousarray(Wq[sl].T),
            "wkT": np.ascontiguousarray(Wk[sl].T),
            "wvT": np.ascontiguousarray(Wv[sl].T),
            "woT": woT,
            "bq": np.ascontiguousarray(bq[sl].reshape(NH, 128).T),
            "bk": np.ascontiguousarray(bk[sl].reshape(NH, 128).T),
            "bv": np.ascontiguousarray(bv[sl].reshape(1, HDc)),
            "bo": bo_r,
            "ones": _ONES,
        })
    return in_maps


def gather_outputs(cfg: Cfg, results):
    """results: list of per-core {'out': (NH*128, E)} -> full (N, S, E)."""
    E = cfg.E
    full = np.empty((N_BATCH, SEQ, E), np.float32)
    cores_per_batch = N_CORES // N_BATCH
    rows = cfg.NH * 128
    for c in range(N_CORES):
        n = c // cores_per_batch
        r0 = (c % cores_per_batch) * rows
        full[n, r0:r0 + rows, :] = results[c]["out"]
    return full


_CACHE = {}


def kernel(**inputs) -> np.ndarray:
    from concourse.bass_utils import run_bass_kernel_spmd
    cfg = Cfg()
    if "nc" not in _CACHE:
        _CACHE["nc"] = build_program(cfg)
    nc = _CACHE["nc"]
    in_maps = shard_inputs(cfg, **inputs)
    res = run_bass_kernel_spmd(nc, in_maps, core_ids=list(range(N_CORES)))
    return gather_outputs(cfg, res.results)
